# revision 41
# baseline (speedup 1.0000x reference)
"""ButterflyLinear Trainium2 kernel.

Math: out[b, s, i] = (sum_o x[b, s, o] * W[o, i]) * mask[s, i], with
mask[s, i] = 1 iff 4s <= i < 4s+4 (stride-4 band). The band makes the
output block-diagonal: s-rows [128t, 128t+128) only touch output columns
[512t, 512t+512) -- an 8x compute reduction vs the full matmul.

Sharding (8 cores): core t owns s-block t for all 16 batches
(tensor-parallel split of W columns; no inter-core communication).

The kernel is wire-bound (per-core HBM ~330-390 GB/s; 2MB fp8 x + 1MB
fp16 W in, 0.5MB fp16 out), so everything streams once, overlapped:
  - x ships as float8 e3m4: the PE computes fp8 x fp16 products exactly
    into fp32 PSUM; the only error is the host-side cast (band rel err
    1.32e-2 vs the 2e-2 gate; W in fp8 too measures 1.75e-2 and bought
    no time -- its 1KB-line DMAs run the stream slower).
  - h-major schedule: PSUM bank h accumulates sub-block h (32 s-rows x
    16 batches) over the 8 o-chunks; banks are processed in PAIRS with
    the c-loop outermost so consecutive matmuls alternate banks
    (back-to-back same-bank accumulation measured 259ns/MM vs 215ns
    alternating). Each pair's banks evacuate concurrently (Vector even
    bank, Scalar odd bank -- different banks, parallel PSUM reads) and
    their 128KB output DMAs ride the wire while pair 1 still streams.
  - Ring S (sync) carries w0 first (the first LDWEIGHTS gate must not
    sit behind ScalarE's ~1.3us ACT-table load), then all x in
    consumption order, with w2/w3 dropped in just ahead of pair 1 so W
    bytes never race ahead of the x the PE needs now. Only w1 rides
    ring A early; ring A otherwise carries the output DMAs. Every DMA
    is a fully contiguous >=2KB-line HBM block (one dram tensor per
    transfer; 512B-line DMAs measured latency-bound at ~100 GB/s).
  - 11 dummy matmuls on a zeroed tile bridge from kernel start to the
    first real matmul's data (~4.7us), so the PE HAM clock-gate is at
    2.4GHz when real work arrives and never re-throttles (a mid-kernel
    PE idle gap >=2.5us measured a re-throttle and 630ns cold MMs).
  - DMA completion sems lag the last data byte by ~2us (receipt round
    trip), so the last x chunk is small (128KB) and the tail is kept
    short: last MM -> paired evacs -> out3 split across both rings.

Host extracts the 4-wide diagonal from the [n, (g, m)] blocks into the
zero-filled (16, 1024, 4096) result.
"""

import sys
from contextlib import ExitStack

import numpy as np

if "/opt/trn_rl_repo" not in sys.path:
    sys.path.insert(0, "/opt/trn_rl_repo")

import concourse.bass as bass  # noqa: E402,F401
import concourse.tile as tile  # noqa: E402
from concourse import bacc, mybir  # noqa: E402
from concourse.bass_utils import run_bass_kernel_spmd  # noqa: E402

B = 16  # batch
NT = 8  # s-blocks == cores
SB = 128  # s rows per block / pack rows per group
NC_ = 8  # o chunks
KC = 128  # o rows per chunk
NI = 512  # output columns per block
QB = 4  # batches packed per group
RW = SB // QB  # s-rows per sub-block (32)
NH = QB  # sub-blocks per s-block
NW = 4 * RW  # W window per sub-block (128)
NG = B // QB  # batch groups (4)
M = NG * SB  # moving free dim (512)

X_DT = mybir.dt.float8e3  # e3m4
W_DT = mybir.dt.float16
F32 = mybir.dt.float32
OUT_DT = mybir.dt.float16

# x DMA blocks: lists of (h, c0, c1) merged into one contiguous transfer.
# Bank-pair matmuls gate on both banks' chunks anyway, so pair-merged
# 512KB DMAs have the same gating times as per-h 256KB ones while using
# 5 fewer issue slots / semaphore lanes (end-of-stream semaphore crowding
# measured 2-3us of receipt lag). The final chunk stays small (128KB).
# All x rides ring S in consumption order (moving later-needed x to the
# other ring steals bandwidth from the bytes the PE needs now -- measured
# +2.4us). The final chunk stays small (128KB) for a short last
# completion-semaphore lag.
XBLOCKS = [
    [(0, 0, 4), (1, 0, 4)],
    [(0, 4, 8), (1, 4, 8)],
    [(2, 0, 4), (3, 0, 4)],
    [(2, 4, 8), (3, 4, 6)],
    [(3, 6, 8)],
]
XRING_A = set()

_STATE: dict = {}


def _build():
    if "nc" in _STATE:
        return _STATE["nc"]

    nc = bacc.Bacc("TRN2", target_bir_lowering=False, debug=False, num_devices=NT)
    # x{b}[p, seg, m] = x[4g + qi, 128t + 32h + r, 128c + p],
    #   m = 128g + 32qi + r; segments enumerate XBLOCKS[b]'s (h, c) pairs.
    xts = {
        b: nc.dram_tensor(
            f"x{b}",
            [KC, sum(c1 - c0 for _, c0, c1 in blk) * M],
            X_DT,
            kind="ExternalInput",
        ).ap()
        for b, blk in enumerate(XBLOCKS)
    }
    # w{h}[p, c, n] = W[128c + p, 512t + 128h + n]
    wts = {
        h: nc.dram_tensor(f"w{h}", [KC, NC_ * NW], W_DT, kind="ExternalInput").ap()
        for h in range(NH)
    }
    # out[h, n, (g, m)] = ps[h][n, 128g + m]
    out = nc.dram_tensor("out", [NH, NW, M], OUT_DT, kind="ExternalOutput").ap()

    with tile.TileContext(nc) as tc, ExitStack() as ctx:
        wp = ctx.enter_context(tc.tile_pool(name="w", bufs=1))
        xp = ctx.enter_context(tc.tile_pool(name="x", bufs=1))
        pp = ctx.enter_context(tc.tile_pool(name="ps", bufs=5, space="PSUM"))
        op = ctx.enter_context(tc.tile_pool(name="o", bufs=1))

        # HAM warm-up: dummy PE work with no input deps bridges from kernel
        # start until the first real matmul's data lands, pushing the
        # clock-gate to 2.4GHz before real work arrives.
        dm = op.tile([KC, M], X_DT, tag="dm")
        nc.gpsimd.memset(dm[:], 0)
        psd = pp.tile([NW, M], F32, tag="ps", name="ps_dummy")
        for _ in range(11):
            nc.tensor.matmul(psd[:], dm[:, 0:NW], dm[:], start=True, stop=True)

        chunk = {}
        wc = {}

        def xdma(b):
            blk = XBLOCKS[b]
            t = xp.tile(
                [KC, sum(c1 - c0 for _, c0, c1 in blk) * M], X_DT, tag=f"x{b}"
            )
            eng = nc.scalar if b in XRING_A else nc.sync
            eng.dma_start(out=t[:], in_=xts[b])
            seg = 0
            for h, c0, c1 in blk:
                for c in range(c0, c1):
                    chunk[(h, c)] = (t, seg)
                    seg += 1

        def wdma(h, eng):
            t = wp.tile([KC, NC_ * NW], W_DT, tag=f"w{h}")
            eng.dma_start(out=t[:], in_=wts[h])
            wc[h] = t

        # S-ring issue order (== arrival order == consumption order).
        wdma(0, nc.sync)
        wdma(1, nc.scalar)
        # ScalarE's ACT-table load is compiler-inserted just before its
        # first ACTIVATE -- this tiny copy placed AFTER the w1 issue
        # triggers the ~1.3us load now without blocking it, and keeps it
        # out of the evacuation path later.
        warm = op.tile([KC, 2], F32, tag="warm")
        nc.scalar.copy(warm[:], dm[:, 0:2])
        xdma(0)
        xdma(1)
        wdma(2, nc.sync)
        wdma(3, nc.sync)
        xdma(2)
        xdma(3)
        xdma(4)

        ps = [pp.tile([NW, M], F32, tag="ps", name=f"ps_{h}") for h in range(NH)]
        ot = [
            op.tile([NW, M], OUT_DT, tag=f"ot{h}", name=f"ot_{h}")
            for h in range(NH)
        ]

        def xmov(h, c):
            t, seg = chunk[(h, c)]
            return t[:, seg * M : (seg + 1) * M]

        def wsl(h, c):
            return wc[h][:, c * NW : (c + 1) * NW]

        for hp in (0, 2):
            # c-outer bank alternation, except the last two c of the last
            # pair run h-grouped -- the final x block's semaphore then
            # gates only bank 3's last two matmuls (3 gated MMs -> 2),
            # and bank 2 closes a slot earlier for its evacuation.
            if hp == 0:
                order = [(c, h) for c in range(NC_) for h in (hp, hp + 1)]
            else:
                order = [(c, h) for c in range(NC_ - 2) for h in (hp, hp + 1)]
                order += [(6, hp), (7, hp), (6, hp + 1), (7, hp + 1)]
            for c, h in order:
                nc.tensor.matmul(
                    ps[h][:, :],
                    wsl(h, c),
                    xmov(h, c),
                    start=(c == 0),
                    stop=(c == NC_ - 1),
                )
            nc.vector.tensor_copy(ot[hp][:], ps[hp][:])
            nc.scalar.copy(ot[hp + 1][:], ps[hp + 1][:])
            if hp == 0:
                nc.scalar.dma_start(out=out[0], in_=ot[0][:])
                nc.scalar.dma_start(out=out[1], in_=ot[1][:])
            else:
                nc.scalar.dma_start(out=out[2], in_=ot[2][:])
                nc.scalar.dma_start(out=out[3, 0:64], in_=ot[3][0:64, :])
                nc.sync.dma_start(out=out[3, 64:128], in_=ot[3][64:128, :])

    nc.compile()
    _STATE["nc"] = nc
    return nc


def _shard(x, W):
    x = np.ascontiguousarray(np.asarray(x, dtype=np.float32)).astype(mybir.dt.np(X_DT))
    W = np.ascontiguousarray(np.asarray(W, dtype=np.float32)).astype(mybir.dt.np(W_DT))
    # x[b, s, o] -> xr[g, qi, t, h, r, c, p] -> [t, h, p, c, (g, qi, r)]
    xr = x.reshape(NG, QB, NT, NH, RW, NC_, KC)
    xts = np.transpose(xr, (2, 3, 6, 5, 0, 1, 4)).reshape(NT, NH, KC, NC_, M)
    # W[o, i] -> wr[c, p, t, h, n] -> [t, h, p, c, n]
    wr = W.reshape(NC_, KC, NT, NH, NW)
    wts = np.transpose(wr, (2, 3, 1, 0, 4))
    maps = []
    for t in range(NT):
        m = {}
        for b, blk in enumerate(XBLOCKS):
            m[f"x{b}"] = np.ascontiguousarray(
                np.concatenate(
                    [xts[t, h, :, c0:c1] for h, c0, c1 in blk], axis=1
                ).reshape(KC, -1)
            )
        for h in range(NH):
            m[f"w{h}"] = np.ascontiguousarray(wts[t, h].reshape(KC, NC_ * NW))
        maps.append(m)
    return maps


def kernel(x, W, _trace=False, _trace_kwargs=None):
    nc = _build()
    in_maps = _shard(x, W)
    res = run_bass_kernel_spmd(
        nc,
        in_maps,
        list(range(NT)),
        trace=_trace,
        **(_trace_kwargs or {}),
    )
    _STATE["last_run"] = res
    band = np.empty((B, NT * SB, 4), dtype=np.float32)
    r_idx = np.arange(RW)
    for t in range(NT):
        blk4 = np.ascontiguousarray(
            res.results[t]["out"].astype(np.float32)
        )  # [h, n, 512]
        for h in range(NH):
            blk = blk4[h]  # [n=128, (g, m)=512]
            e = blk.strides[1]
            # value (g, qi, r, j) sits at blk[4r + j, 128g + 32qi + r]
            v = np.lib.stride_tricks.as_strided(
                blk,
                shape=(NG, QB, RW, 4),
                strides=(128 * e, 32 * e, blk.strides[0] * 4 + e, blk.strides[0]),
            )
            # [g, qi, r, j] -> b = 4g + qi, s = 128t + 32h + r
            band[:, 128 * t + 32 * h + r_idx, :] = v.reshape(B, RW, 4)
    s_idx = np.arange(NT * SB)
    y = np.zeros((B, NT * SB, NT * SB, 4), dtype=np.float32)
    y[:, s_idx, s_idx, :] = band
    return y.reshape(B, NT * SB, NT * NI)


# revision 42
# speedup vs baseline: 1.0094x; 1.0094x over previous
"""ButterflyLinear Trainium2 kernel.

Math: out[b, s, i] = (sum_o x[b, s, o] * W[o, i]) * mask[s, i], with
mask[s, i] = 1 iff 4s <= i < 4s+4 (stride-4 band). The band makes the
output block-diagonal: s-rows [128t, 128t+128) only touch output columns
[512t, 512t+512) -- an 8x compute reduction vs the full matmul.

Sharding (8 cores): core t owns s-block t for all 16 batches
(tensor-parallel split of W columns; no inter-core communication).

The kernel is wire-bound (per-core HBM ~330-390 GB/s; 2MB fp8 x + 1MB
fp16 W in, 0.5MB fp16 out), so everything streams once, overlapped:
  - x ships as float8 e3m4: the PE computes fp8 x fp16 products exactly
    into fp32 PSUM; the only error is the host-side cast (band rel err
    1.32e-2 vs the 2e-2 gate; W in fp8 too measures 1.75e-2 and bought
    no time -- its 1KB-line DMAs run the stream slower).
  - h-major schedule: PSUM bank h accumulates sub-block h (32 s-rows x
    16 batches) over the 8 o-chunks; banks are processed in PAIRS with
    the c-loop outermost so consecutive matmuls alternate banks
    (back-to-back same-bank accumulation measured 259ns/MM vs 215ns
    alternating). Each pair's banks evacuate concurrently (Vector even
    bank, Scalar odd bank -- different banks, parallel PSUM reads) and
    their 128KB output DMAs ride the wire while pair 1 still streams.
  - Ring S (sync) carries w0 first (the first LDWEIGHTS gate must not
    sit behind ScalarE's ~1.3us ACT-table load), then all x in
    consumption order, with w2/w3 dropped in just ahead of pair 1 so W
    bytes never race ahead of the x the PE needs now. Only w1 rides
    ring A early; ring A otherwise carries the output DMAs. Every DMA
    is a fully contiguous >=2KB-line HBM block (one dram tensor per
    transfer; 512B-line DMAs measured latency-bound at ~100 GB/s).
  - 11 dummy matmuls on a zeroed tile bridge from kernel start to the
    first real matmul's data (~4.7us), so the PE HAM clock-gate is at
    2.4GHz when real work arrives and never re-throttles (a mid-kernel
    PE idle gap >=2.5us measured a re-throttle and 630ns cold MMs).
  - DMA completion sems lag the last data byte by ~2us (receipt round
    trip), so the last x chunk is small (128KB) and the tail is kept
    short: last MM -> paired evacs -> out3 split across both rings.

Host extracts the 4-wide diagonal from the [n, (g, m)] blocks into the
zero-filled (16, 1024, 4096) result.
"""

import sys
from contextlib import ExitStack

import numpy as np

if "/opt/trn_rl_repo" not in sys.path:
    sys.path.insert(0, "/opt/trn_rl_repo")

import concourse.bass as bass  # noqa: E402,F401
import concourse.tile as tile  # noqa: E402
from concourse import bacc, mybir  # noqa: E402
from concourse.bass_utils import run_bass_kernel_spmd  # noqa: E402

B = 16  # batch
NT = 8  # s-blocks == cores
SB = 128  # s rows per block / pack rows per group
NC_ = 8  # o chunks
KC = 128  # o rows per chunk
NI = 512  # output columns per block
QB = 4  # batches packed per group
RW = SB // QB  # s-rows per sub-block (32)
NH = QB  # sub-blocks per s-block
NW = 4 * RW  # W window per sub-block (128)
NG = B // QB  # batch groups (4)
M = NG * SB  # moving free dim (512)

X_DT = mybir.dt.float8e3  # e3m4
W_DT = mybir.dt.float16
F32 = mybir.dt.float32
OUT_DT = mybir.dt.float16

# x DMA blocks: lists of (h, c0, c1) merged into one contiguous transfer.
# Bank-pair matmuls gate on both banks' chunks anyway, so pair-merged
# 512KB DMAs have the same gating times as per-h 256KB ones while using
# 5 fewer issue slots / semaphore lanes (end-of-stream semaphore crowding
# measured 2-3us of receipt lag). The final chunk stays small (128KB).
# All x rides ring S in consumption order (moving later-needed x to the
# other ring steals bandwidth from the bytes the PE needs now -- measured
# +2.4us). The final chunk stays small (128KB) for a short last
# completion-semaphore lag.
XBLOCKS = [
    [(0, 0, 4), (1, 0, 4)],
    [(0, 4, 8), (1, 4, 8)],
    [(2, 0, 4), (3, 0, 4)],
    [(2, 4, 8), (3, 4, 6)],
    [(3, 6, 8)],
]
XRING_A = set()

_STATE: dict = {}


def _build():
    if "nc" in _STATE:
        return _STATE["nc"]

    nc = bacc.Bacc("TRN2", target_bir_lowering=False, debug=False, num_devices=NT)
    # x{b}[p, seg, m] = x[4g + qi, 128t + 32h + r, 128c + p],
    #   m = 128g + 32qi + r; segments enumerate XBLOCKS[b]'s (h, c) pairs.
    xts = {
        b: nc.dram_tensor(
            f"x{b}",
            [KC, sum(c1 - c0 for _, c0, c1 in blk) * M],
            X_DT,
            kind="ExternalInput",
        ).ap()
        for b, blk in enumerate(XBLOCKS)
    }
    # w{h}[p, c, n] = W[128c + p, 512t + 128h + n]
    wts = {
        h: nc.dram_tensor(f"w{h}", [KC, NC_ * NW], W_DT, kind="ExternalInput").ap()
        for h in range(NH)
    }
    # out[h, n, (g, m)] = ps[h][n, 128g + m]
    out = nc.dram_tensor("out", [NH, NW, M], OUT_DT, kind="ExternalOutput").ap()

    with tile.TileContext(nc) as tc, ExitStack() as ctx:
        wp = ctx.enter_context(tc.tile_pool(name="w", bufs=1))
        xp = ctx.enter_context(tc.tile_pool(name="x", bufs=1))
        pp = ctx.enter_context(tc.tile_pool(name="ps", bufs=5, space="PSUM"))
        op = ctx.enter_context(tc.tile_pool(name="o", bufs=1))

        # HAM warm-up: dummy PE work with no input deps bridges from kernel
        # start until the first real matmul's data lands, pushing the
        # clock-gate to 2.4GHz before real work arrives.
        dm = op.tile([KC, M], X_DT, tag="dm")
        nc.gpsimd.memset(dm[:], 0)
        psd = pp.tile([NW, M], F32, tag="ps", name="ps_dummy")
        for _ in range(11):
            nc.tensor.matmul(psd[:], dm[:, 0:NW], dm[:], start=True, stop=True)

        chunk = {}
        wc = {}

        def xdma(b):
            blk = XBLOCKS[b]
            t = xp.tile(
                [KC, sum(c1 - c0 for _, c0, c1 in blk) * M], X_DT, tag=f"x{b}"
            )
            eng = nc.scalar if b in XRING_A else nc.sync
            eng.dma_start(out=t[:], in_=xts[b])
            seg = 0
            for h, c0, c1 in blk:
                for c in range(c0, c1):
                    chunk[(h, c)] = (t, seg)
                    seg += 1

        def wdma(h, eng):
            t = wp.tile([KC, NC_ * NW], W_DT, tag=f"w{h}")
            eng.dma_start(out=t[:], in_=wts[h])
            wc[h] = t

        # S-ring issue order (== arrival order == consumption order).
        wdma(0, nc.sync)
        wdma(1, nc.scalar)
        # ScalarE's ACT-table load is compiler-inserted just before its
        # first ACTIVATE -- this tiny copy placed AFTER the w1 issue
        # triggers the ~1.3us load now without blocking it, and keeps it
        # out of the evacuation path later.
        warm = op.tile([KC, 2], F32, tag="warm")
        nc.scalar.copy(warm[:], dm[:, 0:2])
        xdma(0)
        xdma(1)
        wdma(2, nc.sync)
        wdma(3, nc.sync)
        xdma(2)
        xdma(3)
        xdma(4)

        ps = [pp.tile([NW, M], F32, tag="ps", name=f"ps_{h}") for h in range(NH)]
        ot = [
            op.tile([NW, M], OUT_DT, tag=f"ot{h}", name=f"ot_{h}")
            for h in range(NH)
        ]

        def xmov(h, c):
            t, seg = chunk[(h, c)]
            return t[:, seg * M : (seg + 1) * M]

        def wsl(h, c):
            return wc[h][:, c * NW : (c + 1) * NW]

        for hp in (0, 2):
            for c in range(NC_):
                for h in (hp, hp + 1):
                    nc.tensor.matmul(
                        ps[h][:, :],
                        wsl(h, c),
                        xmov(h, c),
                        start=(c == 0),
                        stop=(c == NC_ - 1),
                    )
            nc.vector.tensor_copy(ot[hp][:], ps[hp][:])
            nc.scalar.copy(ot[hp + 1][:], ps[hp + 1][:])
            if hp == 0:
                nc.scalar.dma_start(out=out[0], in_=ot[0][:])
                nc.scalar.dma_start(out=out[1], in_=ot[1][:])
            else:
                nc.scalar.dma_start(out=out[2], in_=ot[2][:])
                nc.scalar.dma_start(out=out[3, 0:64], in_=ot[3][0:64, :])
                nc.sync.dma_start(out=out[3, 64:128], in_=ot[3][64:128, :])

    nc.compile()
    _STATE["nc"] = nc
    return nc


def _shard(x, W):
    x = np.ascontiguousarray(np.asarray(x, dtype=np.float32)).astype(mybir.dt.np(X_DT))
    W = np.ascontiguousarray(np.asarray(W, dtype=np.float32)).astype(mybir.dt.np(W_DT))
    # x[b, s, o] -> xr[g, qi, t, h, r, c, p] -> [t, h, p, c, (g, qi, r)]
    xr = x.reshape(NG, QB, NT, NH, RW, NC_, KC)
    xts = np.transpose(xr, (2, 3, 6, 5, 0, 1, 4)).reshape(NT, NH, KC, NC_, M)
    # W[o, i] -> wr[c, p, t, h, n] -> [t, h, p, c, n]
    wr = W.reshape(NC_, KC, NT, NH, NW)
    wts = np.transpose(wr, (2, 3, 1, 0, 4))
    maps = []
    for t in range(NT):
        m = {}
        for b, blk in enumerate(XBLOCKS):
            m[f"x{b}"] = np.ascontiguousarray(
                np.concatenate(
                    [xts[t, h, :, c0:c1] for h, c0, c1 in blk], axis=1
                ).reshape(KC, -1)
            )
        for h in range(NH):
            m[f"w{h}"] = np.ascontiguousarray(wts[t, h].reshape(KC, NC_ * NW))
        maps.append(m)
    return maps


def kernel(x, W, _trace=False, _trace_kwargs=None):
    nc = _build()
    in_maps = _shard(x, W)
    res = run_bass_kernel_spmd(
        nc,
        in_maps,
        list(range(NT)),
        trace=_trace,
        **(_trace_kwargs or {}),
    )
    _STATE["last_run"] = res
    band = np.empty((B, NT * SB, 4), dtype=np.float32)
    r_idx = np.arange(RW)
    for t in range(NT):
        blk4 = np.ascontiguousarray(
            res.results[t]["out"].astype(np.float32)
        )  # [h, n, 512]
        for h in range(NH):
            blk = blk4[h]  # [n=128, (g, m)=512]
            e = blk.strides[1]
            # value (g, qi, r, j) sits at blk[4r + j, 128g + 32qi + r]
            v = np.lib.stride_tricks.as_strided(
                blk,
                shape=(NG, QB, RW, 4),
                strides=(128 * e, 32 * e, blk.strides[0] * 4 + e, blk.strides[0]),
            )
            # [g, qi, r, j] -> b = 4g + qi, s = 128t + 32h + r
            band[:, 128 * t + 32 * h + r_idx, :] = v.reshape(B, RW, 4)
    s_idx = np.arange(NT * SB)
    y = np.zeros((B, NT * SB, NT * SB, 4), dtype=np.float32)
    y[:, s_idx, s_idx, :] = band
    return y.reshape(B, NT * SB, NT * NI)


# revision 43
# speedup vs baseline: 1.0371x; 1.0275x over previous
"""ButterflyLinear Trainium2 kernel.

Math: out[b, s, i] = (sum_o x[b, s, o] * W[o, i]) * mask[s, i], with
mask[s, i] = 1 iff 4s <= i < 4s+4 (stride-4 band). The band makes the
output block-diagonal: s-rows [128t, 128t+128) only touch output columns
[512t, 512t+512) -- an 8x compute reduction vs the full matmul.

Sharding (8 cores): core t owns s-block t for all 16 batches
(tensor-parallel split of W columns; no inter-core communication).

The kernel is wire-bound (per-core HBM ~330-390 GB/s; 2MB fp8 x + 1MB
fp16 W in, 0.5MB fp16 out), so everything streams once, overlapped:
  - x ships as float8 e3m4: the PE computes fp8 x fp16 products exactly
    into fp32 PSUM; the only error is the host-side cast (band rel err
    1.32e-2 vs the 2e-2 gate; W in fp8 too measures 1.75e-2 and bought
    no time -- its 1KB-line DMAs run the stream slower).
  - h-major schedule: PSUM bank h accumulates sub-block h (32 s-rows x
    16 batches) over the 8 o-chunks; banks are processed in PAIRS with
    the c-loop outermost so consecutive matmuls alternate banks
    (back-to-back same-bank accumulation measured 259ns/MM vs 215ns
    alternating). Each pair's banks evacuate concurrently (Vector even
    bank, Scalar odd bank -- different banks, parallel PSUM reads) and
    their 128KB output DMAs ride the wire while pair 1 still streams.
  - Ring S (sync) carries w0 first (the first LDWEIGHTS gate must not
    sit behind ScalarE's ~1.3us ACT-table load), then all x in
    consumption order, with w2/w3 dropped in just ahead of pair 1 so W
    bytes never race ahead of the x the PE needs now. Only w1 rides
    ring A early; ring A otherwise carries the output DMAs. Every DMA
    is a fully contiguous >=2KB-line HBM block (one dram tensor per
    transfer; 512B-line DMAs measured latency-bound at ~100 GB/s).
  - 11 dummy matmuls on a zeroed tile bridge from kernel start to the
    first real matmul's data (~4.7us), so the PE HAM clock-gate is at
    2.4GHz when real work arrives and never re-throttles (a mid-kernel
    PE idle gap >=2.5us measured a re-throttle and 630ns cold MMs).
  - DMA completion sems lag the last data byte by ~2us (receipt round
    trip), so the last x chunk is small (128KB) and the tail is kept
    short: last MM -> paired evacs -> out3 split across both rings.

Host extracts the 4-wide diagonal from the [n, (g, m)] blocks into the
zero-filled (16, 1024, 4096) result.
"""

import sys
from contextlib import ExitStack

import numpy as np

if "/opt/trn_rl_repo" not in sys.path:
    sys.path.insert(0, "/opt/trn_rl_repo")

import concourse.bass as bass  # noqa: E402,F401
import concourse.tile as tile  # noqa: E402
from concourse import bacc, mybir  # noqa: E402
from concourse.bass_utils import run_bass_kernel_spmd  # noqa: E402

B = 16  # batch
NT = 8  # s-blocks == cores
SB = 128  # s rows per block / pack rows per group
NC_ = 8  # o chunks
KC = 128  # o rows per chunk
NI = 512  # output columns per block
QB = 4  # batches packed per group
RW = SB // QB  # s-rows per sub-block (32)
NH = QB  # sub-blocks per s-block
NW = 4 * RW  # W window per sub-block (128)
NG = B // QB  # batch groups (4)
M = NG * SB  # moving free dim (512)

X_DT = mybir.dt.float8e3  # e3m4
W_DT = mybir.dt.float16
F32 = mybir.dt.float32
OUT_DT = mybir.dt.float16

# x DMA blocks: lists of (h, c0, c1) merged into one contiguous transfer.
# Bank-pair matmuls gate on both banks' chunks anyway, so pair-merged
# 512KB DMAs have the same gating times as per-h 256KB ones while using
# 5 fewer issue slots / semaphore lanes (end-of-stream semaphore crowding
# measured 2-3us of receipt lag). The final chunk stays small (128KB).
# All x rides ring S in consumption order (moving later-needed x to the
# other ring steals bandwidth from the bytes the PE needs now -- measured
# +2.4us). The final chunk stays small (128KB) for a short last
# completion-semaphore lag.
XBLOCKS = [
    [(0, 0, 4), (1, 0, 4)],
    [(0, 4, 8), (1, 4, 8)],
    [(2, 0, 4), (3, 0, 4)],
    [(2, 4, 8), (3, 4, 6)],
    [(3, 6, 8)],
]
XRING_A = set()

_STATE: dict = {}


def _build():
    if "nc" in _STATE:
        return _STATE["nc"]

    nc = bacc.Bacc("TRN2", target_bir_lowering=False, debug=False, num_devices=NT)
    # x{b}[p, seg, m] = x[4g + qi, 128t + 32h + r, 128c + p],
    #   m = 128g + 32qi + r; segments enumerate XBLOCKS[b]'s (h, c) pairs.
    xts = {
        b: nc.dram_tensor(
            f"x{b}",
            [KC, sum(c1 - c0 for _, c0, c1 in blk) * M],
            X_DT,
            kind="ExternalInput",
        ).ap()
        for b, blk in enumerate(XBLOCKS)
    }
    # w{h}[p, c, n] = W[128c + p, 512t + 128h + n]
    wts = {
        h: nc.dram_tensor(f"w{h}", [KC, NC_ * NW], W_DT, kind="ExternalInput").ap()
        for h in range(NH)
    }
    # out[h, n, (g, m)] = ps[h][n, 128g + m]
    out = nc.dram_tensor("out", [NH, NW, M], OUT_DT, kind="ExternalOutput").ap()

    with tile.TileContext(nc) as tc, ExitStack() as ctx:
        wp = ctx.enter_context(tc.tile_pool(name="w", bufs=1))
        xp = ctx.enter_context(tc.tile_pool(name="x", bufs=1))
        pp = ctx.enter_context(tc.tile_pool(name="ps", bufs=5, space="PSUM"))
        op = ctx.enter_context(tc.tile_pool(name="o", bufs=1))

        # HAM warm-up: dummy PE work with no input deps bridges from kernel
        # start until the first real matmul's data lands, pushing the
        # clock-gate to 2.4GHz before real work arrives.
        dm = op.tile([KC, M], X_DT, tag="dm")
        nc.gpsimd.memset(dm[:], 0)
        psd = pp.tile([NW, M], F32, tag="ps", name="ps_dummy")
        # 14 dummies (~6us cold): sized for SLOW HBM phases, where the
        # first x semaphore can slip to ~14us -- an underrun there idles
        # the PE, HAM re-throttles, and 8 real MMs measured cold (634ns),
        # compounding the slow phase by ~1.6us. In fast phases the ~1us
        # overhang is absorbed by the PE's catch-up margin over the
        # stream before pair 0 completes.
        for _ in range(14):
            nc.tensor.matmul(psd[:], dm[:, 0:NW], dm[:], start=True, stop=True)

        chunk = {}
        wc = {}

        def xdma(b):
            blk = XBLOCKS[b]
            t = xp.tile(
                [KC, sum(c1 - c0 for _, c0, c1 in blk) * M], X_DT, tag=f"x{b}"
            )
            eng = nc.scalar if b in XRING_A else nc.sync
            eng.dma_start(out=t[:], in_=xts[b])
            seg = 0
            for h, c0, c1 in blk:
                for c in range(c0, c1):
                    chunk[(h, c)] = (t, seg)
                    seg += 1

        def wdma(h, eng):
            t = wp.tile([KC, NC_ * NW], W_DT, tag=f"w{h}")
            eng.dma_start(out=t[:], in_=wts[h])
            wc[h] = t

        # S-ring issue order (== arrival order == consumption order).
        wdma(0, nc.sync)
        wdma(1, nc.scalar)
        # ScalarE's ACT-table load is compiler-inserted just before its
        # first ACTIVATE -- this tiny copy placed AFTER the w1 issue
        # triggers the ~1.3us load now without blocking it, and keeps it
        # out of the evacuation path later.
        warm = op.tile([KC, 2], F32, tag="warm")
        nc.scalar.copy(warm[:], dm[:, 0:2])
        xdma(0)
        xdma(1)
        wdma(2, nc.sync)
        wdma(3, nc.sync)
        xdma(2)
        xdma(3)
        xdma(4)

        ps = [pp.tile([NW, M], F32, tag="ps", name=f"ps_{h}") for h in range(NH)]
        ot = [
            op.tile([NW, M], OUT_DT, tag=f"ot{h}", name=f"ot_{h}")
            for h in range(NH)
        ]

        def xmov(h, c):
            t, seg = chunk[(h, c)]
            return t[:, seg * M : (seg + 1) * M]

        def wsl(h, c):
            return wc[h][:, c * NW : (c + 1) * NW]

        for hp in (0, 2):
            for c in range(NC_):
                for h in (hp, hp + 1):
                    nc.tensor.matmul(
                        ps[h][:, :],
                        wsl(h, c),
                        xmov(h, c),
                        start=(c == 0),
                        stop=(c == NC_ - 1),
                    )
            nc.vector.tensor_copy(ot[hp][:], ps[hp][:])
            nc.scalar.copy(ot[hp + 1][:], ps[hp + 1][:])
            if hp == 0:
                nc.scalar.dma_start(out=out[0], in_=ot[0][:])
                nc.scalar.dma_start(out=out[1], in_=ot[1][:])
            else:
                nc.scalar.dma_start(out=out[2], in_=ot[2][:])
                nc.scalar.dma_start(out=out[3, 0:64], in_=ot[3][0:64, :])
                nc.sync.dma_start(out=out[3, 64:128], in_=ot[3][64:128, :])

    nc.compile()
    _STATE["nc"] = nc
    return nc


def _shard(x, W):
    x = np.ascontiguousarray(np.asarray(x, dtype=np.float32)).astype(mybir.dt.np(X_DT))
    W = np.ascontiguousarray(np.asarray(W, dtype=np.float32)).astype(mybir.dt.np(W_DT))
    # x[b, s, o] -> xr[g, qi, t, h, r, c, p] -> [t, h, p, c, (g, qi, r)]
    xr = x.reshape(NG, QB, NT, NH, RW, NC_, KC)
    xts = np.transpose(xr, (2, 3, 6, 5, 0, 1, 4)).reshape(NT, NH, KC, NC_, M)
    # W[o, i] -> wr[c, p, t, h, n] -> [t, h, p, c, n]
    wr = W.reshape(NC_, KC, NT, NH, NW)
    wts = np.transpose(wr, (2, 3, 1, 0, 4))
    maps = []
    for t in range(NT):
        m = {}
        for b, blk in enumerate(XBLOCKS):
            m[f"x{b}"] = np.ascontiguousarray(
                np.concatenate(
                    [xts[t, h, :, c0:c1] for h, c0, c1 in blk], axis=1
                ).reshape(KC, -1)
            )
        for h in range(NH):
            m[f"w{h}"] = np.ascontiguousarray(wts[t, h].reshape(KC, NC_ * NW))
        maps.append(m)
    return maps


def kernel(x, W, _trace=False, _trace_kwargs=None):
    nc = _build()
    in_maps = _shard(x, W)
    res = run_bass_kernel_spmd(
        nc,
        in_maps,
        list(range(NT)),
        trace=_trace,
        **(_trace_kwargs or {}),
    )
    _STATE["last_run"] = res
    band = np.empty((B, NT * SB, 4), dtype=np.float32)
    r_idx = np.arange(RW)
    for t in range(NT):
        blk4 = np.ascontiguousarray(
            res.results[t]["out"].astype(np.float32)
        )  # [h, n, 512]
        for h in range(NH):
            blk = blk4[h]  # [n=128, (g, m)=512]
            e = blk.strides[1]
            # value (g, qi, r, j) sits at blk[4r + j, 128g + 32qi + r]
            v = np.lib.stride_tricks.as_strided(
                blk,
                shape=(NG, QB, RW, 4),
                strides=(128 * e, 32 * e, blk.strides[0] * 4 + e, blk.strides[0]),
            )
            # [g, qi, r, j] -> b = 4g + qi, s = 128t + 32h + r
            band[:, 128 * t + 32 * h + r_idx, :] = v.reshape(B, RW, 4)
    s_idx = np.arange(NT * SB)
    y = np.zeros((B, NT * SB, NT * SB, 4), dtype=np.float32)
    y[:, s_idx, s_idx, :] = band
    return y.reshape(B, NT * SB, NT * NI)
